# revision 32
# baseline (speedup 1.0000x reference)
"""Trainium2 Bass kernel for causal self-attention (dense transformer block attn).

Reference computation (per batch b):
    qkv = x @ W_attn + b_attn ; split into per-head Q, K, V (16 heads, hs=64)
    att = softmax(mask(Q K^T / sqrt(hs))) ; y = att @ V ; out = y @ W_proj + b_proj

Sharding (8 cores): data parallel on B (2) x tensor parallel on head groups
(4 groups of 4 heads, Megatron-style column/row split of W_attn / W_proj).
Each core computes a partial out^T [1024, 2048] (bf16); host sums the 4
partials per batch, adds b_proj and transposes.

Core kernel layout notes:
  - Everything on-chip is transposed: x^T, q/k^T ([feature, T]), scores are
    computed as S^T = K Q^T with k-positions on partitions so that the PV
    matmul needs no transposes (P^T is the moving operand, V natural the
    stationary).
  - V is produced directly in natural [key, feature] layout by swapping the
    matmul roles (stationary = x^T k-chunk, moving = W_v columns); its bias
    is a rank-1 matmul (ones[1,128] x bv[1,128]) prepended to the chain.
    This removes all PE transposes and their DVE evacuation copies.
  - Emission is flash-style (outer loop over 512-wide q blocks, inner over
    128-wide k chunks). The ACT exp() stream is the pacing resource, so the
    schedule works to never stall it: Q/K projections for block qb+1 and
    V-natural rounds run as PE fillers inside block qb, and PSUM pools are
    split (scores / qkv+proj / pv) so no iteration-boundary matmul ever
    waits on the previous iteration's DVE backlog (which also kept HAM
    re-throttling the PE clock).
  - Softmax denominator: the PV stationary is [V | ones] (or [ones | V]) so
    the complementary 64 psum partitions accumulate copies of
    sum_k P[q,k]; a single-row reciprocal_approx_fast + a DRAM partition
    broadcast bounce (on the otherwise idle gpsimd SWDGE queue) yields the
    per-q scale; one DVE multiply per head normalizes during evacuation.
  - exp() runs on ScalarE straight out of PSUM in wide [128, 2, <=512]
    instructions (two heads at once) to amortize the ~352-cycle ACT
    overhead.
"""

import numpy as np
import ml_dtypes

import concourse.bass as bass
import concourse.tile as tile
import concourse.mybir as mybir
from concourse import bacc
from concourse.bass_utils import run_bass_kernel_spmd

BF16 = mybir.dt.bfloat16
F32 = mybir.dt.float32
AF = mybir.ActivationFunctionType

T = 2048          # sequence length
C = 1024          # model dim
HPC = 4           # heads per core
HS = 64           # head size
NF = 3 * HPC * HS  # per-core qkv features (768)
N_CORES = 8
QB = 512          # q block (psum bank of f32)

bf16 = ml_dtypes.bfloat16


def build_kernel():
    nc = bacc.Bacc("TRN2", target_bir_lowering=False, debug=False)

    xT = nc.dram_tensor("xT", [C, T], BF16, kind="ExternalInput").ap()
    W = nc.dram_tensor("W", [C, NF], BF16, kind="ExternalInput").ap()
    bcols = nc.dram_tensor("bcols", [128, 4], F32, kind="ExternalInput").ap()
    bv = nc.dram_tensor("bv", [1, 256], BF16, kind="ExternalInput").ap()
    Wp = nc.dram_tensor("Wp", [HPC * HS, C], BF16, kind="ExternalInput").ap()
    mask = nc.dram_tensor("mask", [128, 128], BF16, kind="ExternalInput").ap()
    outT = nc.dram_tensor("outT", [C, T], BF16, kind="ExternalOutput").ap()

    with tile.TileContext(nc) as tc:
        _emit(nc, tc, xT, W, bcols, bv, Wp, mask, outT)
    nc.compile()
    return nc


def _emit(nc, tc, xT, W, bcols, bv, Wp, mask, outT):
    from contextlib import ExitStack

    ctx = ExitStack()
    consts = ctx.enter_context(tc.tile_pool(name="consts", bufs=1))
    pt_pool = ctx.enter_context(tc.tile_pool(name="pt", bufs=1))
    rt_pool = ctx.enter_context(tc.tile_pool(name="rt", bufs=2))
    osb_pool = ctx.enter_context(tc.tile_pool(name="osb", bufs=2))
    ps_s = ctx.enter_context(tc.tile_pool(name="ps_s", bufs=2, space="PSUM"))
    ps_sm = ctx.enter_context(tc.tile_pool(name="ps_sm", bufs=2, space="PSUM"))
    ps_pv = ctx.enter_context(tc.tile_pool(name="ps_pv", bufs=2, space="PSUM"))

    # ---------------- constant / input loads ----------------
    # x and W interleaved per c-chunk with x split in T halves so the first
    # q blocks are available early; big/first loads on the Sync HWDGE queue,
    # second x half on the Scalar HWDGE queue, small consts on gpsimd SWDGE.
    xT_v = xT.rearrange("(c p) t -> p c t", p=128)
    xT_t = consts.tile([128, 8, T], BF16, tag="xT", name="xT_t")
    W_v = W.rearrange("(c p) n -> p c n", p=128)
    W_t = consts.tile([128, 8, NF], BF16, tag="W", name="W_t")
    # priority order on two HWDGE queues (one queue alone sustains only
    # ~150GB/s on these 1KB-line transfers): the critical set (W cols for
    # Q/K-pair0 + x q-block 0, which unblocks the exp stream) is split
    # even/odd across both queues, then later-needed data in use order.
    for c in range(8):
        eng = nc.sync if c % 2 == 0 else nc.scalar
        eng.dma_start(out=W_t[:, c, 0:128], in_=W_v[:, c, 0:128])
        eng.dma_start(out=W_t[:, c, 256:384], in_=W_v[:, c, 256:384])
        eng.dma_start(out=xT_t[:, c, 0:QB], in_=xT_v[:, c, 0:QB])
    for c in range(8):
        eng = nc.sync if c % 2 == 0 else nc.scalar
        eng.dma_start(out=W_t[:, c, 128:256], in_=W_v[:, c, 128:256])
        eng.dma_start(out=W_t[:, c, 384:NF], in_=W_v[:, c, 384:NF])
    for c in range(8):
        # blocks 1..3 in one wide transfer per c-chunk (3KB lines sustain
        # much better DMA throughput than 1KB quarters)
        eng = nc.sync if c % 2 == 0 else nc.scalar
        eng.dma_start(out=xT_t[:, c, QB:T], in_=xT_v[:, c, QB:T])
    b_t = consts.tile([128, 4], F32, tag="b", name="b_t")
    nc.gpsimd.dma_start(out=b_t, in_=bcols)
    bv_t = consts.tile([1, 256], BF16, tag="bv", name="bv_t")
    nc.gpsimd.dma_start(out=bv_t, in_=bv)
    Wp_t = consts.tile([128, 2, C], BF16, tag="Wp", name="Wp_t")
    nc.gpsimd.dma_start(out=Wp_t, in_=Wp.rearrange("(k p) n -> p k n", p=128))
    mask_t = consts.tile([128, 128], BF16, tag="mask", name="mask_t")
    nc.gpsimd.dma_start(out=mask_t, in_=mask)

    qkvT = consts.tile([128, 4, T], BF16, tag="qkvT", name="qkvT")
    # vnat[p, pair, jc, hl, col]: PV stationary tiles. hl=0: [V | ones],
    # hl=1: [ones | V] so that y lands on the partitions matching yT layout.
    vnat = consts.tile([128, 2, 16, 2, 128], BF16, tag="vnat", name="vnat")
    yT = consts.tile([128, 2, T], BF16, tag="yT", name="yT")
    ones1 = consts.tile([1, 128], BF16, tag="ones1", name="ones1")
    nc.vector.memset(ones1, 1.0)
    # full-partition ones column block: 1-partition slices of it are the
    # stationaries for the rank-1 denominator partition-broadcast matmuls
    onesc = consts.tile([128, 64], BF16, tag="onesc", name="onesc")
    nc.vector.memset(onesc, 1.0)

    # warm up the ACT exp table early so the ~2.7us load overlaps the lead-in
    warm = consts.tile([128, 8], F32, tag="warm", name="warm")
    nc.vector.memset(warm, 0.0)
    nc.scalar.activation(warm, warm, AF.Exp, scale=1.0)

    # input-DMA-independent junk matmuls: keep the PE array fed during the
    # initial input-streaming window so HAM un-throttles before real work.
    # The junk psum shares the "pv" slots: all junk writes are emitted in
    # the first iteration, before any pv tile cycles onto its slot.
    jw = consts.tile([128, QB], BF16, tag="jw", name="jw")
    nc.vector.memset(jw, 0.0)
    junk = ps_pv.tile([128, QB], F32, tag="pv", name="junk")

    def keep_warm(n=2):
        for _ in range(n):
            nc.tensor.matmul(junk, lhsT=jw[:, 0:128], rhs=jw, start=True,
                             stop=True)

    nc.vector.memset(vnat[:, :, :, 0, 64:128], 1.0)
    nc.vector.memset(vnat[:, :, :, 1, 0:64], 1.0)

    # ---------------- phase helpers ----------------
    def qkv_part(nf, qb4):
        # one q block of q/k^T[nf*128:(nf+1)*128, :]  (+ bias on evac)
        ps = ps_sm.tile([128, QB], F32, tag="sm", name="ps_qkv")
        for c in range(8):
            nc.tensor.matmul(
                ps,
                lhsT=W_t[:, c, nf * 128:(nf + 1) * 128],
                rhs=xT_t[:, c, qb4 * QB:(qb4 + 1) * QB],
                start=(c == 0),
                stop=(c == 7),
            )
        nc.vector.tensor_scalar_add(
            qkvT[:, nf, qb4 * QB:(qb4 + 1) * QB], ps, b_t[:, nf:nf + 1]
        )

    def vn_batch(p, kb):
        # V natural for pair p, key chunks 4*kb..4*kb+3, batched into one
        # psum bank: per chunk a rank-1 bias matmul + 8 c-chunk matmuls
        # (only the very first matmul clears the bank; later chunks land on
        # has_written-clear regions so they overwrite), then a single
        # 4D-strided copy psum -> vnat[:, p, kc, hl, 64*hl : 64*hl+64].
        ps = ps_sm.tile([128, 4, 128], F32, tag="sm", name="ps_vn")
        for r in range(4):
            kc = 4 * kb + r
            nc.tensor.matmul(ps[:, r, :], lhsT=ones1,
                             rhs=bv_t[0:1, 128 * p:128 * p + 128],
                             start=(r == 0), stop=False,
                             skip_group_check=True)
            for c in range(8):
                nc.tensor.matmul(
                    ps[:, r, :],
                    lhsT=xT_t[:, c, kc * 128:(kc + 1) * 128],
                    rhs=W_t[:, c, 512 + 128 * p:512 + 128 * p + 128],
                    start=False,
                    stop=(r == 3 and c == 7),
                    skip_group_check=True,
                )
        v0 = vnat[:, p, 4 * kb, 0, 0:64]
        dst = bass.AP(tensor=v0.tensor, offset=v0.offset,
                      ap=[v0.ap[0], [256, 4], [192, 2], [1, 64]])
        s0 = ps[:, 0, 0:64]
        src = bass.AP(tensor=s0.tensor, offset=s0.offset,
                      ap=[s0.ap[0], [128, 4], [64, 2], [1, 64]])
        nc.vector.tensor_copy(dst, src)

    pt_tiles = {}

    def s_part(p, j, qb4):
        # scores^T + exp for pair p, key chunk j, q block qb4 (both heads)
        wj = T - 128 * j
        if (p, j) not in pt_tiles:
            pt_tiles[(p, j)] = pt_pool.tile(
                [128, 2, wj], BF16, tag=f"pt{j}",
                name=f"pt_{p}_{j}", bufs=2 if j < 2 else 1)
        pt = pt_tiles[(p, j)]
        qlo = max(128 * j, QB * qb4)
        qhi = QB * (qb4 + 1)
        lo = qlo - QB * qb4
        ps = ps_s.tile([128, 2, QB], F32, tag="s", name="ps_s_t")
        for hl in range(2):
            nc.tensor.matmul(
                ps[:, hl, lo:QB],
                lhsT=qkvT[64 * hl:64 * hl + 64, 2 + p, j * 128:(j + 1) * 128],
                rhs=qkvT[64 * hl:64 * hl + 64, p, qlo:qhi],
                start=True,
                stop=True,
            )
        nc.scalar.activation(
            pt[:, :, (qlo - 128 * j):(qhi - 128 * j)],
            ps[:, :, lo:QB],
            AF.Exp,
            scale=0.125,
        )
        if j // 4 == qb4:
            # zero the q < k upper triangle of the diagonal chunk (both heads
            # in one mul via a broadcast AP over the head dim). Runs on the
            # otherwise-idle GpSimd engine to keep the DVE queue short.
            mb = bass.AP(tensor=mask_t.tensor, offset=mask_t.offset,
                         ap=[mask_t.ap[0], [0, 2], [1, 128]])
            nc.gpsimd.tensor_mul(pt[:, :, 0:128], pt[:, :, 0:128], mb)

    sb_tiles = {}
    rt2_tiles = {}
    pv_ps = {}

    def pv_mms(p, hl, qb4, jlo, jhi, start, stop):
        ps = pv_ps[(p, hl)]
        for jp in range(jlo, jhi + 1):
            pt = pt_tiles[(p, jp)]
            qlo = max(qb4 * QB, 128 * jp)
            qhi = qb4 * QB + QB
            nc.tensor.matmul(
                ps[:, (qlo - qb4 * QB):(qhi - qb4 * QB)],
                lhsT=vnat[:, p, jp, hl, :],
                rhs=pt[:, hl, (qlo - 128 * jp):(qhi - 128 * jp)],
                start=(start and jp == jlo),
                stop=(stop and jp == jhi),
            )

    def pv_rect(p, qb4):
        # below-diagonal part of both heads' PV chains: reads only pt data
        # from earlier iterations, so it can go first in the iteration with
        # no fresh cross-engine deps. Leaves the psum accumulation open.
        for hl in range(2):
            pv_ps[(p, hl)] = ps_pv.tile([128, QB], F32, tag="pv",
                                        name=f"ps_pv{p}{hl}")
        if qb4 > 0:
            for hl in range(2):
                pv_mms(p, hl, qb4, 0, 4 * qb4 - 1, start=True, stop=False)

    def pv_diag(p, qb4):
        # diagonal-block chunks (their exp+mask land late in the previous
        # iteration's ACT/DVE queues, so this pops a few score steps in),
        # then evac: the y rows to f32 SBUF, plus one representative
        # denominator-copy row to bf16 (the moving operand of pv_norm's
        # rank-1 partition-broadcast matmul).
        db = rt_pool.tile([128, 2, QB], BF16, tag="db", name="db", bufs=2)
        for hl in range(2):
            pv_mms(p, hl, qb4, 4 * qb4, 4 * qb4 + 3,
                   start=(qb4 == 0), stop=True)
            drow = 64 - 64 * hl  # one representative denominator-copy row
            ysl = slice(64 * hl, 64 * hl + 64)
            ps = pv_ps.pop((p, hl))
            sb = rt_pool.tile([128, QB], F32, tag="sb", name="sb", bufs=6)
            nc.vector.tensor_copy(sb[ysl, :], ps[ysl, :])
            nc.vector.tensor_copy(db[drow:drow + 1, hl, :],
                                  ps[drow:drow + 1, :])
            sb_tiles[(p, hl, qb4)] = sb
        rt2_tiles[(p, qb4)] = db

    def pv_norm(p, qb4):
        # normalize both heads' y into yT. Two rank-1 matmuls broadcast the
        # bf16 denominator rows across partitions into one psum bank (hl0's
        # row, at partition 64, lands on partitions 0:64 and vice versa) —
        # all on-chip, replacing a ~10us DRAM partition-broadcast bounce.
        # Then one full-tile reciprocal (single-partition reciprocal_approx
        # is broken) and one DVE multiply per head.
        qsl = slice(qb4 * QB, (qb4 + 1) * QB)
        db = rt2_tiles.pop((p, qb4))
        ps_rc = ps_sm.tile([128, QB], F32, tag="sm", name="ps_rc")
        # two independent start/stop groups: the second start only clears
        # has_written bits, the first group's DATA persists in the bank
        # (a chained start/stop pair across col groups miscomputes on HW)
        nc.tensor.matmul(ps_rc[0:64, :], lhsT=onesc[64:65, 0:64],
                         rhs=db[64:65, 0, :], start=True, stop=True,
                         skip_group_check=True)
        nc.tensor.matmul(ps_rc[64:128, :], lhsT=onesc[0:1, 0:64],
                         rhs=db[0:1, 1, :], start=True, stop=True,
                         skip_group_check=True)
        rc = rt_pool.tile([128, QB], F32, tag="rc", name="rc", bufs=2)
        nc.vector.reciprocal_approx_fast(out=rc, in_=ps_rc)
        for hl in range(2):
            ysl = slice(64 * hl, 64 * hl + 64)
            sb = sb_tiles.pop((p, hl, qb4))
            nc.vector.tensor_mul(yT[ysl, p, qsl], sb[ysl, :], rc[ysl, :])

    outT_v = outT.rearrange("(n p) t -> p n t", p=128)

    def proj_u(qb4, nf2, evac_engine, pool=None):
        # final projection, one nf2 unit (2 psum rounds + output DMA) of the
        # 4 per q block (needs yT of both pairs for this block). In-loop the
        # psums come from the "pv" slots (free between pv_diag at ~step 3
        # and the next iteration's pv_rect) so the Q/K/vn "sm" slots never
        # wait on proj's DVE evacuations.
        qsl = slice(qb4 * QB, (qb4 + 1) * QB)
        ob = osb_pool.tile([128, 2, QB], BF16, tag="osb", name="ob")
        for sub in range(2):
            nf = nf2 * 2 + sub
            if pool is None:
                ps = ps_pv.tile([128, QB], F32, tag="pv", name="ps_o")
            else:
                ps = ps_sm.tile([128, QB], F32, tag="sm", name="ps_o")
            for kc in range(2):
                nc.tensor.matmul(
                    ps,
                    lhsT=Wp_t[:, kc, nf * 128:(nf + 1) * 128],
                    rhs=yT[:, kc, qsl],
                    start=(kc == 0),
                    stop=(kc == 1),
                )
            if evac_engine == "scalar" or (evac_engine == "mixed" and sub == 1):
                nc.scalar.copy(ob[:, sub, :], ps)
            else:
                nc.vector.tensor_copy(ob[:, sub, :], ps)
        nc.sync.dma_start(out=outT_v[:, nf2 * 2:nf2 * 2 + 2, qsl], in_=ob)

    def proj_qb(qb4, evac_engine):
        for nf2 in range(4):
            proj_u(qb4, nf2, evac_engine, pool="sm")

    # ---------------- emission schedule ----------------
    # flash-style: per 512-wide q block of pair 0 then pair 1: scores+exp
    # for all k chunks <= the diagonal, with carry-over work (lagged
    # PV rect/diag, 2-blocks-lagged normalize, proj) and LOOK-AHEAD work
    # (next block's Q/K projections, V-natural rounds) popped at explicit
    # score steps. Rules encoded here:
    #   - pv_rect(prev) at step 0 (no fresh deps), pv_diag(prev) ~step 3
    #     (its exp+mask retire from the previous iteration's queues by then)
    #   - pv_norm(2-ago) at step 0 so its DVE muls land EARLY in the queue
    #     (its broadcast DMA has been in flight since mid-prev iteration)
    #     and proj of that block can follow in the same iteration.
    #   - Q/K(next) late; their DVE bias-adds still clear before the next
    #     iteration's first score step needs them.
    def iteration(p, qb4, fillers, warm_every=0):
        fill = sorted(fillers, key=lambda sf: sf[0])
        nf_s = 4 * qb4 + 4
        for j in range(nf_s):
            s_part(p, j, qb4)
            if warm_every:
                keep_warm(warm_every)
            while fill and fill[0][0] <= j:
                fill.pop(0)[1]()
        for _, f in fill:
            f()

    def F(fn, *a):
        return lambda: fn(*a)

    QK = qkv_part
    with nc.named_scope("p0"):
        qkv_part(0, 0)   # Q_0 block 0
        qkv_part(2, 0)   # K_0 chunks 0..3
        iteration(0, 0, [(1, F(vn_batch, 0, 0)),
                         (2, F(QK, 0, 1)), (3, F(QK, 2, 1))], warm_every=2)
        iteration(0, 1, [(1, F(pv_rect, 0, 0)), (3, F(pv_diag, 0, 0)),
                         (2, F(vn_batch, 0, 1)),
                         (5, F(QK, 0, 2)), (6, F(QK, 2, 2))])
        iteration(0, 2, [(1, F(pv_rect, 0, 1)), (2, F(pv_norm, 0, 0)),
                         (3, F(pv_diag, 0, 1)), (5, F(vn_batch, 0, 2)),
                         (8, F(QK, 0, 3)), (9, F(QK, 2, 3))])
        iteration(0, 3, [(1, F(pv_rect, 0, 2)), (2, F(pv_norm, 0, 1)),
                         (3, F(pv_diag, 0, 2)), (6, F(vn_batch, 0, 3)),
                         (10, F(QK, 1, 0)), (11, F(QK, 3, 0))])
    with nc.named_scope("p1"):
        iteration(1, 0, [(1, F(pv_rect, 0, 3)), (2, F(pv_norm, 0, 2)),
                         (2, F(pv_diag, 0, 3)), (1, F(vn_batch, 1, 0)),
                         (3, F(QK, 1, 1)), (3, F(QK, 3, 1))])
        iteration(1, 1, [(1, F(pv_rect, 1, 0)), (2, F(pv_norm, 0, 3)),
                         (3, F(pv_diag, 1, 0)), (2, F(vn_batch, 1, 1)),
                         (5, F(QK, 1, 2)), (6, F(QK, 3, 2))])
        iteration(1, 2, [(1, F(pv_rect, 1, 1)), (2, F(pv_norm, 1, 0)),
                         (3, F(pv_diag, 1, 1)), (4, F(vn_batch, 1, 2)),
                         (5, F(proj_u, 0, 0, "mixed")),
                         (6, F(proj_u, 0, 1, "mixed")),
                         (7, F(proj_u, 0, 2, "mixed")),
                         (9, F(proj_u, 0, 3, "mixed")),
                         (10, F(QK, 1, 3)), (11, F(QK, 3, 3))])
        iteration(1, 3, [(1, F(pv_rect, 1, 2)), (2, F(pv_norm, 1, 1)),
                         (3, F(pv_diag, 1, 2)), (4, F(vn_batch, 1, 3)),
                         (5, F(pv_norm, 1, 2)),
                         (6, F(proj_u, 1, 0, "mixed")),
                         (8, F(proj_u, 1, 1, "mixed")),
                         (10, F(proj_u, 1, 2, "mixed")),
                         (12, F(proj_u, 1, 3, "mixed"))])
    with nc.named_scope("tail"):
        pv_rect(1, 3)
        with nc.named_scope("prj2"):
            proj_u(2, 0, "scalar", pool="sm")
            proj_u(2, 1, "scalar", pool="sm")
            proj_u(2, 2, "scalar", pool="sm")
            proj_u(2, 3, "scalar", pool="sm")
        pv_diag(1, 3)
        pv_norm(1, 3)
        with nc.named_scope("prj3"):
            for nf2 in range(4):
                proj_u(3, nf2, "scalar", pool="sm")
    ctx.close()


# ---------------------------------------------------------------------------
# host-side wrapper
# ---------------------------------------------------------------------------

_NC_CACHE = {}


def _get_nc():
    if "nc" not in _NC_CACHE:
        _NC_CACHE["nc"] = build_kernel()
    return _NC_CACHE["nc"]


def make_in_maps(x, W_attn, b_attn, W_proj, b_proj):
    # multiplicative causal mask for the diagonal chunk, [k, q]: 1 where q >= k
    mask_np = np.triu(np.ones((128, 128), np.float32)).astype(bf16)
    in_maps = []
    for core in range(N_CORES):
        b = core // 4
        g = core % 4
        cols = np.r_[256 * g:256 * g + 256,
                     1024 + 256 * g:1024 + 256 * g + 256,
                     2048 + 256 * g:2048 + 256 * g + 256]
        bc = b_attn[cols]
        in_maps.append({
            "xT": np.ascontiguousarray(x[b].T).astype(bf16),
            "W": np.ascontiguousarray(W_attn[:, cols]).astype(bf16),
            "bcols": np.ascontiguousarray(
                bc[0:512].reshape(4, 128).T).astype(np.float32),
            "bv": np.ascontiguousarray(bc[512:768].reshape(1, 256)).astype(bf16),
            "Wp": np.ascontiguousarray(
                W_proj[256 * g:256 * g + 256, :]).astype(bf16),
            "mask": mask_np,
        })
    return in_maps


def kernel(x, W_attn, b_attn, W_proj, b_proj, _trace=False, _trace_kwargs=None):
    x = np.asarray(x, np.float32)
    W_attn = np.asarray(W_attn, np.float32)
    b_attn = np.asarray(b_attn, np.float32)
    W_proj = np.asarray(W_proj, np.float32)
    b_proj = np.asarray(b_proj, np.float32)

    nc = _get_nc()
    in_maps = make_in_maps(x, W_attn, b_attn, W_proj, b_proj)
    res = run_bass_kernel_spmd(
        nc, in_maps, core_ids=list(range(N_CORES)), trace=_trace,
        **(_trace_kwargs or {}),
    )
    B = x.shape[0]
    out = np.zeros((B, T, C), np.float32)
    for core in range(N_CORES):
        b = core // 4
        out[b] += res.results[core]["outT"].T.astype(np.float32)
    out += b_proj[None, None, :]
    if _trace:
        kernel._last_results = res
    return out


if __name__ == "__main__":
    # smoke test: build only
    nc = build_kernel()
    print("built ok")


# revision 33
# speedup vs baseline: 1.1752x; 1.1752x over previous
"""Trainium2 Bass kernel for causal self-attention (dense transformer block attn).

Reference computation (per batch b):
    qkv = x @ W_attn + b_attn ; split into per-head Q, K, V (16 heads, hs=64)
    att = softmax(mask(Q K^T / sqrt(hs))) ; y = att @ V ; out = y @ W_proj + b_proj

Sharding (8 cores): data parallel on B (2) x tensor parallel on head groups
(4 groups of 4 heads, Megatron-style column/row split of W_attn / W_proj).
Each core computes a partial out^T [1024, 2048] (bf16); host sums the 4
partials per batch, adds b_proj and transposes.

Core kernel layout notes:
  - Everything on-chip is transposed: x^T, q/k^T ([feature, T]), scores are
    computed as S^T = K Q^T with k-positions on partitions so that the PV
    matmul needs no transposes (P^T is the moving operand, V natural the
    stationary).
  - V is produced directly in natural [key, feature] layout by swapping the
    matmul roles (stationary = x^T k-chunk, moving = W_v columns); its bias
    is a rank-1 matmul (ones[1,128] x bv[1,128]) prepended to the chain.
    This removes all PE transposes and their DVE evacuation copies.
  - Emission is flash-style (outer loop over 512-wide q blocks, inner over
    128-wide k chunks). The ACT exp() stream is the pacing resource, so the
    schedule works to never stall it: Q/K projections for block qb+1 and
    V-natural rounds run as PE fillers inside block qb, and PSUM pools are
    split (scores / qkv+proj / pv) so no iteration-boundary matmul ever
    waits on the previous iteration's DVE backlog (which also kept HAM
    re-throttling the PE clock).
  - Softmax denominator: the PV stationary is [V | ones] (or [ones | V]) so
    the complementary 64 psum partitions accumulate copies of
    sum_k P[q,k]; a single-row reciprocal_approx_fast + a DRAM partition
    broadcast bounce (on the otherwise idle gpsimd SWDGE queue) yields the
    per-q scale; one DVE multiply per head normalizes during evacuation.
  - exp() runs on ScalarE straight out of PSUM in wide [128, 2, <=512]
    instructions (two heads at once) to amortize the ~352-cycle ACT
    overhead.
"""

import numpy as np
import ml_dtypes

import concourse.bass as bass
import concourse.tile as tile
import concourse.mybir as mybir
from concourse import bacc
from concourse.bass_utils import run_bass_kernel_spmd

BF16 = mybir.dt.bfloat16
F32 = mybir.dt.float32
AF = mybir.ActivationFunctionType

T = 2048          # sequence length
C = 1024          # model dim
HPC = 4           # heads per core
HS = 64           # head size
NF = 3 * HPC * HS  # per-core qkv features (768)
N_CORES = 8
QB = 512          # q block (psum bank of f32)

bf16 = ml_dtypes.bfloat16


def build_kernel():
    nc = bacc.Bacc("TRN2", target_bir_lowering=False, debug=False)

    xT = nc.dram_tensor("xT", [C, T], BF16, kind="ExternalInput").ap()
    W = nc.dram_tensor("W", [C, NF], BF16, kind="ExternalInput").ap()
    bcols = nc.dram_tensor("bcols", [128, 4], F32, kind="ExternalInput").ap()
    bv = nc.dram_tensor("bv", [1, 256], BF16, kind="ExternalInput").ap()
    Wp = nc.dram_tensor("Wp", [HPC * HS, C], BF16, kind="ExternalInput").ap()
    mask = nc.dram_tensor("mask", [128, 128], BF16, kind="ExternalInput").ap()
    outT = nc.dram_tensor("outT", [C, T], BF16, kind="ExternalOutput").ap()

    with tile.TileContext(nc) as tc:
        _emit(nc, tc, xT, W, bcols, bv, Wp, mask, outT)
    nc.compile()
    return nc


def _emit(nc, tc, xT, W, bcols, bv, Wp, mask, outT):
    from contextlib import ExitStack

    ctx = ExitStack()
    consts = ctx.enter_context(tc.tile_pool(name="consts", bufs=1))
    pt_pool = ctx.enter_context(tc.tile_pool(name="pt", bufs=1))
    rt_pool = ctx.enter_context(tc.tile_pool(name="rt", bufs=2))
    osb_pool = ctx.enter_context(tc.tile_pool(name="osb", bufs=2))
    ps_s = ctx.enter_context(tc.tile_pool(name="ps_s", bufs=2, space="PSUM"))
    ps_sm = ctx.enter_context(tc.tile_pool(name="ps_sm", bufs=2, space="PSUM"))
    ps_pv = ctx.enter_context(tc.tile_pool(name="ps_pv", bufs=2, space="PSUM"))

    # ---------------- constant / input loads ----------------
    # x and W interleaved per c-chunk with x split in T halves so the first
    # q blocks are available early; big/first loads on the Sync HWDGE queue,
    # second x half on the Scalar HWDGE queue, small consts on gpsimd SWDGE.
    xT_v = xT.rearrange("(c p) t -> p c t", p=128)
    xT_t = consts.tile([128, 8, T], BF16, tag="xT", name="xT_t")
    W_v = W.rearrange("(c p) n -> p c n", p=128)
    W_t = consts.tile([128, 8, NF], BF16, tag="W", name="W_t")
    # priority order on two HWDGE queues (one queue alone sustains only
    # ~150GB/s on these 1KB-line transfers): the critical set (W cols for
    # Q/K-pair0 + x q-block 0, which unblocks the exp stream) is split
    # even/odd across both queues, then later-needed data in use order.
    for c in range(8):
        eng = nc.sync if c % 2 == 0 else nc.scalar
        eng.dma_start(out=W_t[:, c, 0:384], in_=W_v[:, c, 0:384])
        eng.dma_start(out=xT_t[:, c, 0:QB], in_=xT_v[:, c, 0:QB])
    for c in range(8):
        eng = nc.sync if c % 2 == 0 else nc.scalar
        eng.dma_start(out=W_t[:, c, 384:NF], in_=W_v[:, c, 384:NF])
    for c in range(8):
        # blocks 1..3 in one wide transfer per c-chunk (3KB lines sustain
        # much better DMA throughput than 1KB quarters)
        eng = nc.sync if c % 2 == 0 else nc.scalar
        eng.dma_start(out=xT_t[:, c, QB:T], in_=xT_v[:, c, QB:T])
    b_t = consts.tile([128, 4], F32, tag="b", name="b_t")
    nc.gpsimd.dma_start(out=b_t, in_=bcols)
    bv_t = consts.tile([1, 256], BF16, tag="bv", name="bv_t")
    nc.gpsimd.dma_start(out=bv_t, in_=bv)
    Wp_t = consts.tile([128, 2, C], BF16, tag="Wp", name="Wp_t")
    nc.gpsimd.dma_start(out=Wp_t, in_=Wp.rearrange("(k p) n -> p k n", p=128))
    mask_t = consts.tile([128, 128], BF16, tag="mask", name="mask_t")
    nc.gpsimd.dma_start(out=mask_t, in_=mask)

    qkvT = consts.tile([128, 4, T], BF16, tag="qkvT", name="qkvT")
    # vnat[p, pair, jc, hl, col]: PV stationary tiles. hl=0: [V | ones],
    # hl=1: [ones | V] so that y lands on the partitions matching yT layout.
    vnat = consts.tile([128, 2, 16, 2, 128], BF16, tag="vnat", name="vnat")
    yT = consts.tile([128, 2, T], BF16, tag="yT", name="yT")
    ones1 = consts.tile([1, 128], BF16, tag="ones1", name="ones1")
    nc.vector.memset(ones1, 1.0)
    # full-partition ones column block: 1-partition slices of it are the
    # stationaries for the rank-1 denominator partition-broadcast matmuls
    onesc = consts.tile([128, 64], BF16, tag="onesc", name="onesc")
    nc.vector.memset(onesc, 1.0)

    # warm up the ACT exp table early so the ~2.7us load overlaps the lead-in
    warm = consts.tile([128, 8], F32, tag="warm", name="warm")
    nc.vector.memset(warm, 0.0)
    nc.scalar.activation(warm, warm, AF.Exp, scale=1.0)

    # input-DMA-independent junk matmuls: keep the PE array fed during the
    # initial input-streaming window so HAM un-throttles before real work.
    # The junk psum shares the "pv" slots: all junk writes are emitted in
    # the first iteration, before any pv tile cycles onto its slot.
    jw = consts.tile([128, QB], BF16, tag="jw", name="jw")
    nc.vector.memset(jw, 0.0)
    junk = ps_pv.tile([128, QB], F32, tag="pv", name="junk")

    def keep_warm(n=2):
        for _ in range(n):
            nc.tensor.matmul(junk, lhsT=jw[:, 0:128], rhs=jw, start=True,
                             stop=True)

    nc.vector.memset(vnat[:, :, :, 0, 64:128], 1.0)
    nc.vector.memset(vnat[:, :, :, 1, 0:64], 1.0)

    # ---------------- phase helpers ----------------
    def qkv_part(nf, qb4):
        # one q block of q/k^T[nf*128:(nf+1)*128, :]  (+ bias on evac)
        ps = ps_sm.tile([128, QB], F32, tag="sm", name="ps_qkv")
        for c in range(8):
            nc.tensor.matmul(
                ps,
                lhsT=W_t[:, c, nf * 128:(nf + 1) * 128],
                rhs=xT_t[:, c, qb4 * QB:(qb4 + 1) * QB],
                start=(c == 0),
                stop=(c == 7),
            )
        nc.vector.tensor_scalar_add(
            qkvT[:, nf, qb4 * QB:(qb4 + 1) * QB], ps, b_t[:, nf:nf + 1]
        )

    def vn_batch(p, kb):
        # V natural for pair p, key chunks 4*kb..4*kb+3, batched into one
        # psum bank: per chunk a rank-1 bias matmul + 8 c-chunk matmuls
        # (only the very first matmul clears the bank; later chunks land on
        # has_written-clear regions so they overwrite), then a single
        # 4D-strided copy psum -> vnat[:, p, kc, hl, 64*hl : 64*hl+64].
        ps = ps_sm.tile([128, 4, 128], F32, tag="sm", name="ps_vn")
        for r in range(4):
            kc = 4 * kb + r
            nc.tensor.matmul(ps[:, r, :], lhsT=ones1,
                             rhs=bv_t[0:1, 128 * p:128 * p + 128],
                             start=(r == 0), stop=False,
                             skip_group_check=True)
            for c in range(8):
                nc.tensor.matmul(
                    ps[:, r, :],
                    lhsT=xT_t[:, c, kc * 128:(kc + 1) * 128],
                    rhs=W_t[:, c, 512 + 128 * p:512 + 128 * p + 128],
                    start=False,
                    stop=(r == 3 and c == 7),
                    skip_group_check=True,
                )
        v0 = vnat[:, p, 4 * kb, 0, 0:64]
        dst = bass.AP(tensor=v0.tensor, offset=v0.offset,
                      ap=[v0.ap[0], [256, 4], [192, 2], [1, 64]])
        s0 = ps[:, 0, 0:64]
        src = bass.AP(tensor=s0.tensor, offset=s0.offset,
                      ap=[s0.ap[0], [128, 4], [64, 2], [1, 64]])
        nc.vector.tensor_copy(dst, src)

    pt_tiles = {}

    def s_part(p, j, qb4):
        # scores^T + exp for pair p, key chunk j, q block qb4 (both heads)
        wj = T - 128 * j
        if (p, j) not in pt_tiles:
            pt_tiles[(p, j)] = pt_pool.tile(
                [128, 2, wj], BF16, tag=f"pt{j}",
                name=f"pt_{p}_{j}", bufs=2 if j < 2 else 1)
        pt = pt_tiles[(p, j)]
        qlo = max(128 * j, QB * qb4)
        qhi = QB * (qb4 + 1)
        lo = qlo - QB * qb4
        ps = ps_s.tile([128, 2, QB], F32, tag="s", name="ps_s_t")
        for hl in range(2):
            nc.tensor.matmul(
                ps[:, hl, lo:QB],
                lhsT=qkvT[64 * hl:64 * hl + 64, 2 + p, j * 128:(j + 1) * 128],
                rhs=qkvT[64 * hl:64 * hl + 64, p, qlo:qhi],
                start=True,
                stop=True,
            )
        nc.scalar.activation(
            pt[:, :, (qlo - 128 * j):(qhi - 128 * j)],
            ps[:, :, lo:QB],
            AF.Exp,
            scale=0.125,
        )
        if j // 4 == qb4:
            # zero the q < k upper triangle of the diagonal chunk (both heads
            # in one mul via a broadcast AP over the head dim). Runs on the
            # otherwise-idle GpSimd engine to keep the DVE queue short.
            mb = bass.AP(tensor=mask_t.tensor, offset=mask_t.offset,
                         ap=[mask_t.ap[0], [0, 2], [1, 128]])
            nc.gpsimd.tensor_mul(pt[:, :, 0:128], pt[:, :, 0:128], mb)

    sb_tiles = {}
    rt2_tiles = {}
    pv_ps = {}

    def pv_mms(p, hl, qb4, jlo, jhi, start, stop):
        ps = pv_ps[(p, hl)]
        for jp in range(jlo, jhi + 1):
            pt = pt_tiles[(p, jp)]
            qlo = max(qb4 * QB, 128 * jp)
            qhi = qb4 * QB + QB
            nc.tensor.matmul(
                ps[:, (qlo - qb4 * QB):(qhi - qb4 * QB)],
                lhsT=vnat[:, p, jp, hl, :],
                rhs=pt[:, hl, (qlo - 128 * jp):(qhi - 128 * jp)],
                start=(start and jp == jlo),
                stop=(stop and jp == jhi),
            )

    def pv_rect(p, qb4):
        # below-diagonal part of both heads' PV chains: reads only pt data
        # from earlier iterations, so it can go first in the iteration with
        # no fresh cross-engine deps. Leaves the psum accumulation open.
        for hl in range(2):
            pv_ps[(p, hl)] = ps_pv.tile([128, QB], F32, tag="pv",
                                        name=f"ps_pv{p}{hl}")
        if qb4 > 0:
            for hl in range(2):
                pv_mms(p, hl, qb4, 0, 4 * qb4 - 1, start=True, stop=False)

    def pv_diag(p, qb4):
        # diagonal-block chunks (their exp+mask land late in the previous
        # iteration's ACT/DVE queues, so this pops a few score steps in),
        # then evac: the y rows to f32 SBUF, plus one representative
        # denominator-copy row to bf16 (the moving operand of pv_norm's
        # rank-1 partition-broadcast matmul).
        db = rt_pool.tile([128, 2, QB], BF16, tag="db", name="db", bufs=2)
        for hl in range(2):
            pv_mms(p, hl, qb4, 4 * qb4, 4 * qb4 + 3,
                   start=(qb4 == 0), stop=True)
            drow = 64 - 64 * hl  # one representative denominator-copy row
            ysl = slice(64 * hl, 64 * hl + 64)
            ps = pv_ps.pop((p, hl))
            sb = rt_pool.tile([128, QB], F32, tag="sb", name="sb", bufs=6)
            nc.vector.tensor_copy(sb[ysl, :], ps[ysl, :])
            nc.vector.tensor_copy(db[drow:drow + 1, hl, :],
                                  ps[drow:drow + 1, :])
            sb_tiles[(p, hl, qb4)] = sb
        rt2_tiles[(p, qb4)] = db

    def pv_norm(p, qb4):
        # normalize both heads' y into yT. Two rank-1 matmuls broadcast the
        # bf16 denominator rows across partitions into one psum bank (hl0's
        # row, at partition 64, lands on partitions 0:64 and vice versa) —
        # all on-chip, replacing a ~10us DRAM partition-broadcast bounce.
        # Then one full-tile reciprocal (single-partition reciprocal_approx
        # is broken) and one DVE multiply per head.
        qsl = slice(qb4 * QB, (qb4 + 1) * QB)
        db = rt2_tiles.pop((p, qb4))
        ps_rc = ps_sm.tile([128, QB], F32, tag="sm", name="ps_rc")
        # two independent start/stop groups: the second start only clears
        # has_written bits, the first group's DATA persists in the bank
        # (a chained start/stop pair across col groups miscomputes on HW)
        nc.tensor.matmul(ps_rc[0:64, :], lhsT=onesc[64:65, 0:64],
                         rhs=db[64:65, 0, :], start=True, stop=True,
                         skip_group_check=True)
        nc.tensor.matmul(ps_rc[64:128, :], lhsT=onesc[0:1, 0:64],
                         rhs=db[0:1, 1, :], start=True, stop=True,
                         skip_group_check=True)
        rc = rt_pool.tile([128, QB], F32, tag="rc", name="rc", bufs=2)
        nc.vector.reciprocal_approx_fast(out=rc, in_=ps_rc)
        for hl in range(2):
            ysl = slice(64 * hl, 64 * hl + 64)
            sb = sb_tiles.pop((p, hl, qb4))
            nc.vector.tensor_mul(yT[ysl, p, qsl], sb[ysl, :], rc[ysl, :])

    outT_v = outT.rearrange("(n p) t -> p n t", p=128)

    def proj_u(qb4, nf2, evac_engine, pool=None):
        # final projection, one nf2 unit (2 psum rounds + output DMA) of the
        # 4 per q block (needs yT of both pairs for this block). In-loop the
        # psums come from the "pv" slots (free between pv_diag at ~step 3
        # and the next iteration's pv_rect) so the Q/K/vn "sm" slots never
        # wait on proj's DVE evacuations.
        qsl = slice(qb4 * QB, (qb4 + 1) * QB)
        ob = osb_pool.tile([128, 2, QB], BF16, tag="osb", name="ob")
        for sub in range(2):
            nf = nf2 * 2 + sub
            if pool is None:
                ps = ps_pv.tile([128, QB], F32, tag="pv", name="ps_o")
            else:
                ps = ps_sm.tile([128, QB], F32, tag="sm", name="ps_o")
            for kc in range(2):
                nc.tensor.matmul(
                    ps,
                    lhsT=Wp_t[:, kc, nf * 128:(nf + 1) * 128],
                    rhs=yT[:, kc, qsl],
                    start=(kc == 0),
                    stop=(kc == 1),
                )
            if evac_engine == "scalar" or (evac_engine == "mixed" and sub == 1):
                nc.scalar.copy(ob[:, sub, :], ps)
            else:
                nc.vector.tensor_copy(ob[:, sub, :], ps)
        nc.sync.dma_start(out=outT_v[:, nf2 * 2:nf2 * 2 + 2, qsl], in_=ob)

    def proj_qb(qb4, evac_engine):
        for nf2 in range(4):
            proj_u(qb4, nf2, evac_engine, pool="sm")

    # ---------------- emission schedule ----------------
    # flash-style: per 512-wide q block of pair 0 then pair 1: scores+exp
    # for all k chunks <= the diagonal, with carry-over work (lagged
    # PV rect/diag, 2-blocks-lagged normalize, proj) and LOOK-AHEAD work
    # (next block's Q/K projections, V-natural rounds) popped at explicit
    # score steps. Rules encoded here:
    #   - pv_rect(prev) at step 0 (no fresh deps), pv_diag(prev) ~step 3
    #     (its exp+mask retire from the previous iteration's queues by then)
    #   - pv_norm(2-ago) at step 0 so its DVE muls land EARLY in the queue
    #     (its broadcast DMA has been in flight since mid-prev iteration)
    #     and proj of that block can follow in the same iteration.
    #   - Q/K(next) late; their DVE bias-adds still clear before the next
    #     iteration's first score step needs them.
    def iteration(p, qb4, fillers, warm_every=0):
        fill = sorted(fillers, key=lambda sf: sf[0])
        nf_s = 4 * qb4 + 4
        for j in range(nf_s):
            s_part(p, j, qb4)
            if warm_every:
                keep_warm(warm_every)
            while fill and fill[0][0] <= j:
                fill.pop(0)[1]()
        for _, f in fill:
            f()

    def F(fn, *a):
        return lambda: fn(*a)

    QK = qkv_part
    with nc.named_scope("p0"):
        qkv_part(0, 0)   # Q_0 block 0
        qkv_part(2, 0)   # K_0 chunks 0..3
        iteration(0, 0, [(1, F(vn_batch, 0, 0)),
                         (2, F(QK, 0, 1)), (3, F(QK, 2, 1))], warm_every=2)
        iteration(0, 1, [(1, F(pv_rect, 0, 0)), (3, F(pv_diag, 0, 0)),
                         (2, F(vn_batch, 0, 1)),
                         (5, F(QK, 0, 2)), (6, F(QK, 2, 2))])
        iteration(0, 2, [(1, F(pv_rect, 0, 1)), (2, F(pv_norm, 0, 0)),
                         (3, F(pv_diag, 0, 1)), (5, F(vn_batch, 0, 2)),
                         (8, F(QK, 0, 3)), (9, F(QK, 2, 3))])
        iteration(0, 3, [(1, F(pv_rect, 0, 2)), (2, F(pv_norm, 0, 1)),
                         (3, F(pv_diag, 0, 2)), (6, F(vn_batch, 0, 3)),
                         (10, F(QK, 1, 0)), (11, F(QK, 3, 0))])
    with nc.named_scope("p1"):
        iteration(1, 0, [(1, F(pv_rect, 0, 3)), (2, F(pv_norm, 0, 2)),
                         (2, F(pv_diag, 0, 3)), (1, F(vn_batch, 1, 0)),
                         (3, F(QK, 1, 1)), (3, F(QK, 3, 1))])
        iteration(1, 1, [(1, F(pv_rect, 1, 0)), (2, F(pv_norm, 0, 3)),
                         (3, F(pv_diag, 1, 0)), (2, F(vn_batch, 1, 1)),
                         (5, F(QK, 1, 2)), (6, F(QK, 3, 2))])
        iteration(1, 2, [(1, F(pv_rect, 1, 1)), (2, F(pv_norm, 1, 0)),
                         (3, F(pv_diag, 1, 1)), (4, F(vn_batch, 1, 2)),
                         (5, F(proj_u, 0, 0, "mixed")),
                         (6, F(proj_u, 0, 1, "mixed")),
                         (7, F(proj_u, 0, 2, "mixed")),
                         (9, F(proj_u, 0, 3, "mixed")),
                         (10, F(QK, 1, 3)), (11, F(QK, 3, 3))])
        iteration(1, 3, [(1, F(pv_rect, 1, 2)), (2, F(pv_norm, 1, 1)),
                         (3, F(pv_diag, 1, 2)), (4, F(vn_batch, 1, 3)),
                         (5, F(pv_norm, 1, 2)),
                         (6, F(proj_u, 1, 0, "mixed")),
                         (8, F(proj_u, 1, 1, "mixed")),
                         (10, F(proj_u, 1, 2, "mixed")),
                         (12, F(proj_u, 1, 3, "mixed"))])
    with nc.named_scope("tail"):
        pv_rect(1, 3)
        with nc.named_scope("prj2"):
            proj_u(2, 0, "scalar", pool="sm")
            proj_u(2, 1, "scalar", pool="sm")
            proj_u(2, 2, "scalar", pool="sm")
            proj_u(2, 3, "scalar", pool="sm")
        pv_diag(1, 3)
        pv_norm(1, 3)
        with nc.named_scope("prj3"):
            for nf2 in range(4):
                proj_u(3, nf2, "scalar", pool="sm")
    ctx.close()


# ---------------------------------------------------------------------------
# host-side wrapper
# ---------------------------------------------------------------------------

_NC_CACHE = {}


def _get_nc():
    if "nc" not in _NC_CACHE:
        _NC_CACHE["nc"] = build_kernel()
    return _NC_CACHE["nc"]


def make_in_maps(x, W_attn, b_attn, W_proj, b_proj):
    # multiplicative causal mask for the diagonal chunk, [k, q]: 1 where q >= k
    mask_np = np.triu(np.ones((128, 128), np.float32)).astype(bf16)
    in_maps = []
    for core in range(N_CORES):
        b = core // 4
        g = core % 4
        cols = np.r_[256 * g:256 * g + 256,
                     1024 + 256 * g:1024 + 256 * g + 256,
                     2048 + 256 * g:2048 + 256 * g + 256]
        bc = b_attn[cols]
        in_maps.append({
            "xT": np.ascontiguousarray(x[b].T).astype(bf16),
            "W": np.ascontiguousarray(W_attn[:, cols]).astype(bf16),
            "bcols": np.ascontiguousarray(
                bc[0:512].reshape(4, 128).T).astype(np.float32),
            "bv": np.ascontiguousarray(bc[512:768].reshape(1, 256)).astype(bf16),
            "Wp": np.ascontiguousarray(
                W_proj[256 * g:256 * g + 256, :]).astype(bf16),
            "mask": mask_np,
        })
    return in_maps


def kernel(x, W_attn, b_attn, W_proj, b_proj, _trace=False, _trace_kwargs=None):
    x = np.asarray(x, np.float32)
    W_attn = np.asarray(W_attn, np.float32)
    b_attn = np.asarray(b_attn, np.float32)
    W_proj = np.asarray(W_proj, np.float32)
    b_proj = np.asarray(b_proj, np.float32)

    nc = _get_nc()
    in_maps = make_in_maps(x, W_attn, b_attn, W_proj, b_proj)
    res = run_bass_kernel_spmd(
        nc, in_maps, core_ids=list(range(N_CORES)), trace=_trace,
        **(_trace_kwargs or {}),
    )
    B = x.shape[0]
    out = np.zeros((B, T, C), np.float32)
    for core in range(N_CORES):
        b = core // 4
        out[b] += res.results[core]["outT"].T.astype(np.float32)
    out += b_proj[None, None, :]
    if _trace:
        kernel._last_results = res
    return out


if __name__ == "__main__":
    # smoke test: build only
    nc = build_kernel()
    print("built ok")
